# revision 34
# baseline (speedup 1.0000x reference)
"""GQA attention kernel for Trainium2, 8 NeuronCores.

Sharding: core = b*4 + g  (b = batch 0..1, g = kv-head group 0..3).
Each core handles one batch and one kv group (1 kv head + its 4 query heads).
wq/wo are split by head group (column/row), wk/wv by kv head. The output
projection partial sums (one per group) are reduced on the host.

All matmul operands are bf16 (1 cycle/row on the PE at any output width;
rel-err budget is ~2e-2 so bf16 is plenty). Activations are "transposed"
([feature, seq]) so contractions land on SBUF partitions:
  Q^T[h] = wq_h^T @ x^T, K^T/V^T likewise  (x chunk cached in SBUF, 1 pass)
  S^T    = K_roped^T.T @ Q_roped^T          -> [keys, queries]
  softmax: exp on the Act engine (no max subtraction; scores are O(1)),
           two key-tiles per exp instruction to amortize access latency.
  P@V    : P^T subtile is the STATIONARY operand, V|ones the moving one,
           so out = [q, d+1] and the softmax denominator rides in column
           d for free (no separate ones-matmul).
  O^T    : per-partition reciprocal * scale, then PE transposes back to
           [d, q] for the wo projection.
  y^T    = wo^T @ O^T, staged in SBUF, stored in four DMAs per chunk.
RoPE in [d, s] layout: rope(Z) = Z*C + (Pswap @ Z)*Sg, pair-swap done on
the PE, sign folded into the host-built Sg tile.

The PE is kept fed while the Act engine runs exp by draining a FIFO of
deferred PE work (P@V of the previous head, wo of the previous chunk)
between score matmuls; each FIFO entry carries a PE-time estimate and
each score unit drains roughly its own Act-minus-PE deficit.
"""

import sys

sys.path.insert(0, "/opt/trn_rl_repo")

from collections import deque
from contextlib import ExitStack

import numpy as np
import ml_dtypes

import concourse.bass as bass
import concourse.tile as tile
from concourse import bacc, mybir
from concourse import bass_utils

F32 = mybir.dt.float32
BF = mybir.dt.bfloat16
MULT = mybir.AluOpType.mult
EXP = mybir.ActivationFunctionType.Exp

S = 2048          # sequence length
DM = 2048         # d_model
DH = 128          # head dim
HPC = 4           # query heads per core (= n_rep; one kv group per core)
N_CORES = 8
CH = 512          # query-chunk width (and s-chunk width)
NCHUNK = S // CH  # 4
NT = DM // 128    # 16 contraction tiles of d_model
SCALE = 1.0 / float(np.sqrt(DH))
NPBF = ml_dtypes.bfloat16

_CACHE = {}


def _build():
    nc = bacc.Bacc("TRN2", target_bir_lowering=False, debug=False)

    xT = nc.dram_tensor("xT", [DM, S], BF, kind="ExternalInput").ap()
    wq = nc.dram_tensor("wq", [DM, HPC * DH], BF, kind="ExternalInput").ap()
    wk = nc.dram_tensor("wk", [DM, DH], BF, kind="ExternalInput").ap()
    wv = nc.dram_tensor("wv", [DM, DH], BF, kind="ExternalInput").ap()
    wo = nc.dram_tensor("wo", [HPC * DH, DM], BF, kind="ExternalInput").ap()
    cs = nc.dram_tensor("cs", [DH, S], BF, kind="ExternalInput").ap()
    sn = nc.dram_tensor("sn", [DH, S], BF, kind="ExternalInput").ap()
    pswap = nc.dram_tensor("pswap", [DH, DH], BF, kind="ExternalInput").ap()
    ident = nc.dram_tensor("ident", [128, 128], BF, kind="ExternalInput").ap()
    maskd = nc.dram_tensor("maskd", [128, 128], BF, kind="ExternalInput").ap()
    yT = nc.dram_tensor("yT", [DM, S], BF, kind="ExternalOutput").ap()

    with tile.TileContext(nc) as tc, ExitStack() as ctx:
        consts = ctx.enter_context(tc.tile_pool(name="consts", bufs=1))
        wpool = ctx.enter_context(tc.tile_pool(name="wpool", bufs=1))
        persist = ctx.enter_context(tc.tile_pool(name="persist", bufs=1))
        xpool = ctx.enter_context(tc.tile_pool(name="xpool", bufs=2))
        qpool = ctx.enter_context(tc.tile_pool(name="qpool", bufs=2))
        rawp = ctx.enter_context(tc.tile_pool(name="rawp", bufs=3))
        ropet = ctx.enter_context(tc.tile_pool(name="ropet", bufs=3))
        ppool = ctx.enter_context(tc.tile_pool(name="ppool", bufs=14))
        vtp = ctx.enter_context(tc.tile_pool(name="vtp", bufs=2))
        osbp = ctx.enter_context(tc.tile_pool(name="osbp", bufs=8))
        rdp = ctx.enter_context(tc.tile_pool(name="rdp", bufs=6))
        otsb = ctx.enter_context(tc.tile_pool(name="otsb", bufs=2))
        ystp = ctx.enter_context(tc.tile_pool(name="ystp", bufs=2))
        # PSUM: 8 banks = acc 2x1 + st2 2x2 + ot0/ot1 1x1 each.  Every st2
        # tenant (score pairs, rope pswap, V/O transposes, wo accum) is
        # short-lived so the 2-slot ring never blocks on a long hold.
        ps_acc = ctx.enter_context(tc.tile_pool(name="ps_acc", bufs=2, space="PSUM"))
        ps_st2 = ctx.enter_context(tc.tile_pool(name="ps_st2", bufs=2, space="PSUM"))
        ps_ot = ctx.enter_context(tc.tile_pool(name="ps_ot", bufs=1, space="PSUM"))

        # ---------------- initial DMAs ----------------
        # x chunk 0 on the sync queue, first tile alone so the first Q matmul
        # starts as early as possible; weights on the scalar queue (wo last);
        # small constants via the Pool SWDGE path (doesn't touch HWDGE).
        xT_r = xT.rearrange("(t p) n -> p t n", p=128)
        xc0 = xpool.tile([128, NT, CH], BF, tag="xc")
        wq_r = wq.rearrange("(t p) n -> p t n", p=128)
        wq_sb = wpool.tile([128, NT, HPC * DH], BF, tag="wq")
        splits = [(0, 1), (1, 3), (3, 6), (6, 9), (9, 12), (12, 16)]
        nc.gpsimd.dma_start(xc0[:, 0:1, :], xT_r[:, 0:1, 0:CH])
        nc.scalar.dma_start(wq_sb[:, 0:1, :], wq_r[:, 0:1, :])
        for a, b in splits[1:]:
            nc.sync.dma_start(xc0[:, a:b, :], xT_r[:, a:b, 0:CH])
        for a, b in splits[1:]:
            nc.scalar.dma_start(wq_sb[:, a:b, :], wq_r[:, a:b, :])
        wk_sb = wpool.tile([128, NT, DH], BF, tag="wk")
        nc.scalar.dma_start(wk_sb, wk.rearrange("(t p) n -> p t n", p=128))
        wv_sb = wpool.tile([128, NT, DH], BF, tag="wv")
        nc.scalar.dma_start(wv_sb, wv.rearrange("(t p) n -> p t n", p=128))

        cs_sb = consts.tile([DH, S], BF, tag="cs")
        nc.gpsimd.dma_start(cs_sb, cs)
        sn_sb = consts.tile([DH, S], BF, tag="sn")
        nc.gpsimd.dma_start(sn_sb, sn)
        pswap_sb = consts.tile([DH, DH], BF, tag="pswap")
        nc.gpsimd.dma_start(pswap_sb, pswap)
        ident_sb = consts.tile([128, 128], BF, tag="ident")
        nc.gpsimd.dma_start(ident_sb, ident)
        maskd_sb = consts.tile([128, 128], BF, tag="maskd")
        nc.gpsimd.dma_start(maskd_sb, maskd)

        wo_sb = wpool.tile([128, HPC, DM], BF, tag="wo")
        nc.scalar.dma_start(wo_sb, wo.rearrange("(h p) n -> p h n", p=128))

        kt_sb = persist.tile([DH, S], BF, tag="kt")            # roped K^T
        # V in [s, d] layout + a ones column at d=128 (memset once; the
        # transpose copies only overwrite [:, j, :128] so col 128 stays 1).
        v_sb = persist.tile([128, S // 128, 130], BF, tag="v")
        nc.gpsimd.memset(v_sb, 1.0)

        yT_r = yT.rearrange("(t p) n -> p t n", p=128)

        def rope(raw_ps, c, out_ap):
            """out = raw*C + (Pswap @ raw)*Sg for s-chunk c; raw_ps is PSUM."""
            col = c * CH
            raw_sb = rawp.tile([128, CH], BF, tag="raw")
            nc.vector.tensor_copy(raw_sb, raw_ps)
            sw_ps = ps_st2.tile([128, CH], F32, tag="st2")
            nc.tensor.matmul(sw_ps, pswap_sb, raw_sb, start=True, stop=True)
            ta = ropet.tile([128, CH], BF, tag="ra")
            nc.vector.tensor_tensor(ta, raw_sb, cs_sb[:, col:col + CH], MULT)
            tb = ropet.tile([128, CH], BF, tag="rb")
            nc.vector.tensor_tensor(tb, sw_ps, sn_sb[:, col:col + CH], MULT)
            nc.vector.tensor_add(out_ap, ta, tb)

        # FIFO of (pe_ns, thunk): deferred PE work, drained between score
        # units so the PE stays fed while Act runs exp.
        fifo = deque()
        markers_done = set()

        def drain(budget_ns):
            while fifo and budget_ns > 0:
                ns, thunk = fifo.popleft()
                thunk()
                budget_ns -= max(ns, 1)

        def drain_until(mid):
            """Pop until the marker `mid` has been consumed.  Guarantees the
            P@V consumers of two-heads-ago P tiles are emitted before their
            ring slots are re-allocated (else the scheduler deadlocks)."""
            if isinstance(mid, int) and mid < 0:
                return
            while mid not in markers_done:
                ns, thunk = fifo.popleft()
                thunk()

        xcs = {}
        qts = {}

        def push_qpass(cn, xcn):
            """Queue the Q projection + rope of chunk cn as PE filler for the
            current chunk's attention phase."""
            qt_n = qpool.tile([128, HPC, CH], BF, tag="qt", name=f"qt{cn}")
            qts[cn] = qt_n
            state = {}

            def mk_q_t(pair, t):
                def f():
                    if t == 0:
                        state[pair] = [
                            ps_acc.tile([128, CH], F32, tag="acc",
                                        name=f"accq{cn}_{pair}_{i}")
                            for i in range(2)]
                    for i in range(2):
                        h = 2 * pair + i
                        nc.tensor.matmul(
                            state[pair][i], wq_sb[:, t, h * DH:(h + 1) * DH],
                            xcn[:, t, :], start=(t == 0), stop=(t == NT - 1),
                            skip_group_check=True)
                return f

            def mk_q_rope(pair):
                def f():
                    for i in range(2):
                        rope(state[pair][i], cn, qt_n[:, 2 * pair + i, :])
                return f

            for pair in range(2):
                for t in range(NT):
                    fifo.append((426, mk_q_t(pair, t)))
                fifo.append((800, mk_q_rope(pair)))
            fifo.append((0, lambda mid=("q", cn): markers_done.add(mid)))

        for c in range(NCHUNK):
            col = c * CH

            # ------------ Q projection ------------
            # Chunk 0: inline single 4-head pass (h2/h3 accumulate in the
            # idle st2 slots) keeping the PE abreast of the initial x/wq DMA
            # stream.  Later chunks were queued as FIFO filler during the
            # previous chunk's attention -- just make sure they are emitted.
            if c == 0:
                xc = xc0
                qt_sb = qpool.tile([128, HPC, CH], BF, tag="qt", name="qt0")
                accs = [ps_acc.tile([128, CH], F32, tag="acc", name=f"accq0_{i}")
                        for i in range(2)]
                accs += [ps_st2.tile([128, CH], F32, tag="st2",
                                     name=f"accq0_{i + 2}") for i in range(2)]
                for t in range(NT):
                    for h in range(HPC):
                        nc.tensor.matmul(
                            accs[h], wq_sb[:, t, h * DH:(h + 1) * DH], xc[:, t, :],
                            start=(t == 0), stop=(t == NT - 1))
                for h in (2, 3, 0, 1):   # free the st2 slots first
                    rope(accs[h], c, qt_sb[:, h, :])
            else:
                xc = xcs[c]
                drain_until(("q", c))
                qt_sb = qts[c]

            # prefetch next chunk's x right away (transfers overlap K/V and
            # attention; the slot's previous readers finished last chunk)
            if c + 1 < NCHUNK:
                xn = xpool.tile([128, NT, CH], BF, tag="xc", name=f"xc{c + 1}")
                xcs[c + 1] = xn
                ncol = (c + 1) * CH
                for i in range(4):
                    nc.sync.dma_start(
                        xn[:, 4 * i:4 * i + 4, :],
                        xT_r[:, 4 * i:4 * i + 4, ncol:ncol + CH])

            # ------------ K,V projections (K first: its rope chain is on
            # the critical path to the diagonal score tiles) ------------
            acc_k = ps_acc.tile([128, CH], F32, tag="acc")
            for t in range(NT):
                nc.tensor.matmul(acc_k, wk_sb[:, t, :], xc[:, t, :],
                                 start=(t == 0), stop=(t == NT - 1))
            rope(acc_k, c, kt_sb[:, col:col + CH])
            acc_v = ps_acc.tile([128, CH], F32, tag="acc")
            for t in range(NT):
                nc.tensor.matmul(acc_v, wv_sb[:, t, :], xc[:, t, :],
                                 start=(t == 0), stop=(t == NT - 1))
            vt_sb = vtp.tile([128, CH], BF, tag="vt")
            nc.scalar.copy(vt_sb, acc_v)
            tpv = ps_st2.tile([128, CH], BF, tag="st2", name=f"tpv{c}")
            for u in range(4):
                nc.tensor.matmul(tpv[:, u * 128:(u + 1) * 128],
                                 vt_sb[:, u * 128:(u + 1) * 128], ident_sb,
                                 is_transpose=True, skip_group_check=True)
            nc.vector.tensor_copy(
                v_sb[:, 4 * c:4 * c + 4, :128],
                tpv.rearrange("p (u n) -> p u n", u=4))

            # queue next chunk's Q projection as attention-phase filler
            if c + 1 < NCHUNK:
                push_qpass(c + 1, xcs[c + 1])

            # ------------ attention ------------
            def emit_scores(h):
                """Score+exp+mask for head h. Returns pslice: j -> (ap, base)
                so the P^T column block for (j, qsub u) is ap[:, base+128u:
                base+128(u+1)]."""
                pslice = {}
                # full key tiles, two per exp instruction
                for j in range(0, 4 * c, 2):
                    st2 = ps_st2.tile([128, 2 * CH], F32, tag="st2",
                                      name=f"st{c}_{h}_{j}")
                    nc.tensor.matmul(st2[:, :CH], kt_sb[:, j * 128:(j + 1) * 128],
                                     qt_sb[:, h, :], start=True, stop=True)
                    nc.tensor.matmul(st2[:, CH:], kt_sb[:, (j + 1) * 128:(j + 2) * 128],
                                     qt_sb[:, h, :], start=True, stop=True)
                    pp = ppool.tile([128, 2 * CH], BF, tag="pp",
                                    name=f"pp{c}_{h}_{j}")
                    nc.scalar.activation(pp, st2, EXP, scale=SCALE)
                    pslice[j] = (pp, 0)
                    pslice[j + 1] = (pp, CH)
                    drain(700)
                # diagonal tiles: t=0,1 packed into [0:896], t=2,3 into
                # [0:384] (suffixes shifted left so the exp input is fully
                # written -- no stale-PSUM regions).
                dj = 4 * c
                st2 = ps_st2.tile([128, 896], F32, tag="st2",
                                  name=f"sd01_{c}_{h}")
                nc.tensor.matmul(st2[:, 0:CH], kt_sb[:, dj * 128:(dj + 1) * 128],
                                 qt_sb[:, h, :], start=True, stop=True)
                nc.tensor.matmul(st2[:, CH:896],
                                 kt_sb[:, (dj + 1) * 128:(dj + 2) * 128],
                                 qt_sb[:, h, 128:], start=True, stop=True)
                pp = ppool.tile([128, 896], BF, tag="pd", name=f"pd01_{c}_{h}",
                                bufs=4)
                nc.scalar.activation(pp, st2, EXP, scale=SCALE)
                nc.vector.tensor_tensor(pp[:, 0:128], pp[:, 0:128], maskd_sb, MULT)
                nc.vector.tensor_tensor(pp[:, CH:CH + 128], pp[:, CH:CH + 128],
                                        maskd_sb, MULT)
                pslice[dj] = (pp, 0)
                pslice[dj + 1] = (pp, CH - 128)   # col = 384 + 128u, u >= 1
                drain(650)
                st2 = ps_st2.tile([128, 384], F32, tag="st2",
                                  name=f"sd23_{c}_{h}")
                nc.tensor.matmul(st2[:, 0:256],
                                 kt_sb[:, (dj + 2) * 128:(dj + 3) * 128],
                                 qt_sb[:, h, 256:], start=True, stop=True)
                nc.tensor.matmul(st2[:, 256:384],
                                 kt_sb[:, (dj + 3) * 128:(dj + 4) * 128],
                                 qt_sb[:, h, 384:], start=True, stop=True)
                pp = ppool.tile([128, 384], BF, tag="pd2", name=f"pd23_{c}_{h}",
                                bufs=4)
                nc.scalar.activation(pp, st2, EXP, scale=SCALE)
                nc.vector.tensor_tensor(pp[:, 0:128], pp[:, 0:128], maskd_sb, MULT)
                nc.vector.tensor_tensor(pp[:, 256:384], pp[:, 256:384],
                                        maskd_sb, MULT)
                pslice[dj + 2] = (pp, -256)       # col = -256 + 128u, u >= 2
                pslice[dj + 3] = (pp, -128)       # col = 256 at u == 3
                drain(450)
                return pslice

            def push_pv(h, pslice, ot_sb):
                """Queue P@V + normalize + transpose for head h."""
                ot0 = ps_ot.tile([128, 2, 132], F32, tag="ot0",
                                 name=f"ot0_{c}_{h}")
                ot1 = ps_ot.tile([128, 2, 132], F32, tag="ot1",
                                 name=f"ot1_{c}_{h}")
                osbs = []

                def mk_mm(u, j, first, last):
                    ot = ot0 if u < 2 else ot1
                    p, base = pslice[j]

                    def f():
                        nc.tensor.matmul(
                            ot[:, u % 2, :129],
                            p[:, base + u * 128:base + (u + 1) * 128],
                            v_sb[:, j, :129],
                            start=first, stop=last, skip_group_check=True)
                    return f

                def mk_div(u):
                    ot = ot0 if u < 2 else ot1

                    def f():
                        rd = rdp.tile([128, 1], F32, tag="rd")
                        nc.vector.reciprocal(rd, ot[:, u % 2, 128:129])
                        osb = osbp.tile([128, 128], BF, tag="osb")
                        nc.vector.tensor_scalar_mul(osb, ot[:, u % 2, :128], rd)
                        osbs.append(osb)
                    return f

                def mk_fin(ot_sb=ot_sb):
                    def f():
                        tp2 = ps_st2.tile([128, CH], BF, tag="st2",
                                          name=f"tp2_{c}_{h}")
                        for u in range(4):
                            nc.tensor.matmul(tp2[:, u * 128:(u + 1) * 128],
                                             osbs[u], ident_sb,
                                             is_transpose=True,
                                             skip_group_check=True)
                        if c == NCHUNK - 1 and h % 2 == 0:
                            nc.scalar.copy(ot_sb[:, h, :], tp2)
                        else:
                            nc.vector.tensor_copy(ot_sb[:, h, :], tp2)
                    return f

                for u in range(4):
                    js = list(range(4 * c + u + 1))
                    for idx, j in enumerate(js):
                        fifo.append(
                            (54, mk_mm(u, j, idx == 0, idx == len(js) - 1)))
                    fifo.append((1, mk_div(u)))
                fifo.append((212, mk_fin()))

            ot_sb = otsb.tile([128, HPC, CH], BF, tag="ot", name=f"ot_sb{c}")
            ystage = ystp.tile([128, NT, CH], BF, tag="yst", name=f"yst{c}")
            last = c == NCHUNK - 1
            if last:
                wstg = ystp.tile([128, NT, CH], BF, tag="wstg", bufs=1)

            def mk_wo1(dt_, ot_sb=ot_sb):
                def f():
                    yt_ps = ps_st2.tile([128, CH], F32, tag="st2",
                                        name=f"yt1_{dt_}")
                    for h in range(2):
                        nc.tensor.matmul(
                            yt_ps, wo_sb[:, h, dt_ * 128:(dt_ + 1) * 128],
                            ot_sb[:, h, :], start=(h == 0), stop=(h == 1),
                            skip_group_check=True)
                    nc.vector.tensor_copy(wstg[:, dt_, :], yt_ps)
                return f

            prev = None
            for h in range(HPC):
                gh = c * HPC + h
                drain_until(gh - 2)
                ps = emit_scores(h)
                if prev is not None:
                    push_pv(h - 1, prev, ot_sb)
                    fifo.append((0, lambda mid=gh - 1: markers_done.add(mid)))
                    if last and h == 2:
                        # wo of heads 0/1: extra filler for the final chunk's
                        # h2/h3 score phases (no next-chunk Q pass exists)
                        for dt_ in range(NT):
                            fifo.append((426, mk_wo1(dt_)))
                prev = ps
            push_pv(HPC - 1, prev, ot_sb)
            fifo.append((0, lambda mid=c * HPC + HPC - 1: markers_done.add(mid)))

            # ------------ output projection (queued as filler) ------------
            # One atomic thunk per dt_ tile: the st2 accumulation group must
            # open and close without another st2 alloc slipping in between
            # (the ring has 2 slots; a half-open group would deadlock).
            def mk_wo(dt_, c=c, ot_sb=ot_sb, ystage=ystage):
                def f():
                    yt_ps = ps_st2.tile([128, CH], F32, tag="st2",
                                        name=f"yt{c}_{dt_}")
                    h0 = 2 if c == NCHUNK - 1 else 0
                    for h in range(h0, HPC):
                        nc.tensor.matmul(
                            yt_ps, wo_sb[:, h, dt_ * 128:(dt_ + 1) * 128],
                            ot_sb[:, h, :], start=(h == h0), stop=(h == HPC - 1),
                            skip_group_check=True)
                    if c == NCHUNK - 1:
                        if dt_ % 2 == 0:
                            nc.scalar.tensor_tensor = None  # unused guard
                        nc.vector.tensor_tensor(ystage[:, dt_, :], yt_ps,
                                                wstg[:, dt_, :],
                                                mybir.AluOpType.add)
                    else:
                        nc.vector.tensor_copy(ystage[:, dt_, :], yt_ps)
                return f

            def mk_store(i, w, c=c, ystage=ystage):
                def f():
                    q = nc.sync if c == NCHUNK - 1 else nc.gpsimd
                    q.dma_start(
                        yT_r[:, w * i:w * (i + 1), c * CH:(c + 1) * CH],
                        ystage[:, w * i:w * (i + 1), :])
                return f

            stw = 1 if c == NCHUNK - 1 else 4
            for dt_ in range(NT):
                fifo.append((852, mk_wo(dt_)))
                if dt_ % stw == stw - 1:
                    fifo.append((1, mk_store(dt_ // stw, stw)))

        drain(1 << 30)

    nc.compile()
    return nc


def _host_prep(x, freqs_cos, freqs_sin, wq, wk, wv, wo):
    """Build the 8 per-core input maps (bf16)."""
    cos_t = np.ascontiguousarray(freqs_cos.T)  # [64, S]
    sin_t = np.ascontiguousarray(freqs_sin.T)
    cs = np.repeat(cos_t, 2, axis=0).astype(np.float32)        # [128, S]
    sn = np.repeat(sin_t, 2, axis=0).astype(np.float32)
    sn[0::2] *= -1.0
    cs = cs.astype(NPBF)
    sn = sn.astype(NPBF)

    pswap = np.zeros((DH, DH), dtype=NPBF)
    idx = np.arange(0, DH, 2)
    pswap[idx, idx + 1] = 1.0
    pswap[idx + 1, idx] = 1.0

    ident = np.eye(128, dtype=NPBF)

    # maskd[jj, z] = 1 where z >= jj: the causal triangle of a diagonal
    # 128x128 block (same for every diagonal tile).
    z = np.arange(128)[None, :]
    jj = np.arange(128)[:, None]
    maskd = (z >= jj).astype(NPBF)

    xTs = [np.ascontiguousarray(x[b].T).astype(NPBF) for b in range(2)]
    wq16 = wq.astype(NPBF)
    wk16 = wk.astype(NPBF)
    wv16 = wv.astype(NPBF)
    wo16 = wo.astype(NPBF)

    in_maps = []
    for core in range(N_CORES):
        b, g = divmod(core, HPC)
        in_maps.append({
            "xT": xTs[b],
            "wq": np.ascontiguousarray(wq16[:, g * HPC * DH:(g + 1) * HPC * DH]),
            "wk": np.ascontiguousarray(wk16[:, g * DH:(g + 1) * DH]),
            "wv": np.ascontiguousarray(wv16[:, g * DH:(g + 1) * DH]),
            "wo": np.ascontiguousarray(wo16[g * HPC * DH:(g + 1) * HPC * DH, :]),
            "cs": cs, "sn": sn, "pswap": pswap, "ident": ident, "maskd": maskd,
        })
    return in_maps


def kernel(x, freqs_cos, freqs_sin, mask, wq, wk, wv, wo):
    x = np.asarray(x, dtype=np.float32)
    freqs_cos = np.asarray(freqs_cos, dtype=np.float32)
    freqs_sin = np.asarray(freqs_sin, dtype=np.float32)
    wq = np.asarray(wq, dtype=np.float32)
    wk = np.asarray(wk, dtype=np.float32)
    wv = np.asarray(wv, dtype=np.float32)
    wo = np.asarray(wo, dtype=np.float32)

    if "nc" not in _CACHE:
        _CACHE["nc"] = _build()
    nc = _CACHE["nc"]

    in_maps = _host_prep(x, freqs_cos, freqs_sin, wq, wk, wv, wo)
    res = bass_utils.run_bass_kernel_spmd(nc, in_maps, core_ids=list(range(N_CORES)))

    out = np.empty((2, S, DM), dtype=np.float32)
    for b in range(2):
        acc = res.results[b * HPC]["yT"].astype(np.float32)
        for g in range(1, HPC):
            acc = acc + res.results[b * HPC + g]["yT"].astype(np.float32)
        out[b] = acc.T
    return out


# revision 35
# speedup vs baseline: 1.0278x; 1.0278x over previous
"""GQA attention kernel for Trainium2, 8 NeuronCores.

Sharding: core = b*4 + g  (b = batch 0..1, g = kv-head group 0..3).
Each core handles one batch and one kv group (1 kv head + its 4 query heads).
wq/wo are split by head group (column/row), wk/wv by kv head. The output
projection partial sums (one per group) are reduced on the host.

All matmul operands are bf16 (1 cycle/row on the PE at any output width;
rel-err budget is ~2e-2 so bf16 is plenty). Activations are "transposed"
([feature, seq]) so contractions land on SBUF partitions:
  Q^T[h] = wq_h^T @ x^T, K^T/V^T likewise  (x chunk cached in SBUF, 1 pass)
  S^T    = K_roped^T.T @ Q_roped^T          -> [keys, queries]
  softmax: exp on the Act engine (no max subtraction; scores are O(1)),
           two key-tiles per exp instruction to amortize access latency.
  P@V    : P^T subtile is the STATIONARY operand, V|ones the moving one,
           so out = [q, d+1] and the softmax denominator rides in column
           d for free (no separate ones-matmul).
  O^T    : per-partition reciprocal * scale, then PE transposes back to
           [d, q] for the wo projection.
  y^T    = wo^T @ O^T, staged in SBUF, stored in four DMAs per chunk.
RoPE in [d, s] layout: rope(Z) = Z*C + (Pswap @ Z)*Sg, pair-swap done on
the PE, sign folded into the host-built Sg tile.

The PE is kept fed while the Act engine runs exp by draining a FIFO of
deferred PE work (P@V of the previous head, wo of the previous chunk)
between score matmuls; each FIFO entry carries a PE-time estimate and
each score unit drains roughly its own Act-minus-PE deficit.
"""

import sys

sys.path.insert(0, "/opt/trn_rl_repo")

from collections import deque
from contextlib import ExitStack

import numpy as np
import ml_dtypes

import concourse.bass as bass
import concourse.tile as tile
from concourse import bacc, mybir
from concourse import bass_utils

F32 = mybir.dt.float32
BF = mybir.dt.bfloat16
MULT = mybir.AluOpType.mult
EXP = mybir.ActivationFunctionType.Exp

S = 2048          # sequence length
DM = 2048         # d_model
DH = 128          # head dim
HPC = 4           # query heads per core (= n_rep; one kv group per core)
N_CORES = 8
CH = 512          # query-chunk width (and s-chunk width)
NCHUNK = S // CH  # 4
NT = DM // 128    # 16 contraction tiles of d_model
SCALE = 1.0 / float(np.sqrt(DH))
NPBF = ml_dtypes.bfloat16

_CACHE = {}


def _build():
    nc = bacc.Bacc("TRN2", target_bir_lowering=False, debug=False)

    xT = nc.dram_tensor("xT", [DM, S], BF, kind="ExternalInput").ap()
    wq = nc.dram_tensor("wq", [DM, HPC * DH], BF, kind="ExternalInput").ap()
    wk = nc.dram_tensor("wk", [DM, DH], BF, kind="ExternalInput").ap()
    wv = nc.dram_tensor("wv", [DM, DH], BF, kind="ExternalInput").ap()
    wo = nc.dram_tensor("wo", [HPC * DH, DM], BF, kind="ExternalInput").ap()
    cs = nc.dram_tensor("cs", [DH, S], BF, kind="ExternalInput").ap()
    sn = nc.dram_tensor("sn", [DH, S], BF, kind="ExternalInput").ap()
    pswap = nc.dram_tensor("pswap", [DH, DH], BF, kind="ExternalInput").ap()
    ident = nc.dram_tensor("ident", [128, 128], BF, kind="ExternalInput").ap()
    maskd = nc.dram_tensor("maskd", [128, 128], BF, kind="ExternalInput").ap()
    yT = nc.dram_tensor("yT", [DM, S], BF, kind="ExternalOutput").ap()

    with tile.TileContext(nc) as tc, ExitStack() as ctx:
        consts = ctx.enter_context(tc.tile_pool(name="consts", bufs=1))
        wpool = ctx.enter_context(tc.tile_pool(name="wpool", bufs=1))
        persist = ctx.enter_context(tc.tile_pool(name="persist", bufs=1))
        xpool = ctx.enter_context(tc.tile_pool(name="xpool", bufs=2))
        qpool = ctx.enter_context(tc.tile_pool(name="qpool", bufs=2))
        rawp = ctx.enter_context(tc.tile_pool(name="rawp", bufs=3))
        ropet = ctx.enter_context(tc.tile_pool(name="ropet", bufs=3))
        ppool = ctx.enter_context(tc.tile_pool(name="ppool", bufs=14))
        vtp = ctx.enter_context(tc.tile_pool(name="vtp", bufs=2))
        osbp = ctx.enter_context(tc.tile_pool(name="osbp", bufs=8))
        rdp = ctx.enter_context(tc.tile_pool(name="rdp", bufs=6))
        otsb = ctx.enter_context(tc.tile_pool(name="otsb", bufs=2))
        ystp = ctx.enter_context(tc.tile_pool(name="ystp", bufs=2))
        # PSUM: 8 banks = acc 2x1 + st2 2x2 + ot0/ot1 1x1 each.  Every st2
        # tenant (score pairs, rope pswap, V/O transposes, wo accum) is
        # short-lived so the 2-slot ring never blocks on a long hold.
        ps_acc = ctx.enter_context(tc.tile_pool(name="ps_acc", bufs=2, space="PSUM"))
        ps_st2 = ctx.enter_context(tc.tile_pool(name="ps_st2", bufs=2, space="PSUM"))
        ps_ot = ctx.enter_context(tc.tile_pool(name="ps_ot", bufs=1, space="PSUM"))

        # ---------------- initial DMAs ----------------
        # x chunk 0 on the sync queue, first tile alone so the first Q matmul
        # starts as early as possible; weights on the scalar queue (wo last);
        # small constants via the Pool SWDGE path (doesn't touch HWDGE).
        xT_r = xT.rearrange("(t p) n -> p t n", p=128)
        xc0 = xpool.tile([128, NT, CH], BF, tag="xc")
        wq_r = wq.rearrange("(t p) n -> p t n", p=128)
        wq_sb = wpool.tile([128, NT, HPC * DH], BF, tag="wq")
        splits = [(0, 1), (1, 3), (3, 6), (6, 9), (9, 12), (12, 16)]
        nc.gpsimd.dma_start(xc0[:, 0:1, :], xT_r[:, 0:1, 0:CH])
        nc.scalar.dma_start(wq_sb[:, 0:1, :], wq_r[:, 0:1, :])
        for a, b in splits[1:]:
            nc.sync.dma_start(xc0[:, a:b, :], xT_r[:, a:b, 0:CH])
        for a, b in splits[1:]:
            nc.scalar.dma_start(wq_sb[:, a:b, :], wq_r[:, a:b, :])
        wk_sb = wpool.tile([128, NT, DH], BF, tag="wk")
        nc.scalar.dma_start(wk_sb, wk.rearrange("(t p) n -> p t n", p=128))
        wv_sb = wpool.tile([128, NT, DH], BF, tag="wv")
        nc.scalar.dma_start(wv_sb, wv.rearrange("(t p) n -> p t n", p=128))

        cs_sb = consts.tile([DH, S], BF, tag="cs")
        nc.gpsimd.dma_start(cs_sb, cs)
        sn_sb = consts.tile([DH, S], BF, tag="sn")
        nc.gpsimd.dma_start(sn_sb, sn)
        pswap_sb = consts.tile([DH, DH], BF, tag="pswap")
        nc.gpsimd.dma_start(pswap_sb, pswap)
        ident_sb = consts.tile([128, 128], BF, tag="ident")
        nc.gpsimd.dma_start(ident_sb, ident)
        maskd_sb = consts.tile([128, 128], BF, tag="maskd")
        nc.gpsimd.dma_start(maskd_sb, maskd)

        wo_sb = wpool.tile([128, HPC, DM], BF, tag="wo")
        nc.scalar.dma_start(wo_sb, wo.rearrange("(h p) n -> p h n", p=128))

        kt_sb = persist.tile([DH, S], BF, tag="kt")            # roped K^T
        # V in [s, d] layout + a ones column at d=128 (memset once; the
        # transpose copies only overwrite [:, j, :128] so col 128 stays 1).
        v_sb = persist.tile([128, S // 128, 130], BF, tag="v")
        nc.gpsimd.memset(v_sb, 1.0)

        yT_r = yT.rearrange("(t p) n -> p t n", p=128)

        def rope(raw_ps, c, out_ap):
            """out = raw*C + (Pswap @ raw)*Sg for s-chunk c; raw_ps is PSUM."""
            col = c * CH
            raw_sb = rawp.tile([128, CH], BF, tag="raw")
            nc.vector.tensor_copy(raw_sb, raw_ps)
            sw_ps = ps_st2.tile([128, CH], F32, tag="st2")
            nc.tensor.matmul(sw_ps, pswap_sb, raw_sb, start=True, stop=True)
            ta = ropet.tile([128, CH], BF, tag="ra")
            nc.vector.tensor_tensor(ta, raw_sb, cs_sb[:, col:col + CH], MULT)
            tb = ropet.tile([128, CH], BF, tag="rb")
            nc.vector.tensor_tensor(tb, sw_ps, sn_sb[:, col:col + CH], MULT)
            nc.vector.tensor_add(out_ap, ta, tb)

        # FIFO of (pe_ns, thunk): deferred PE work, drained between score
        # units so the PE stays fed while Act runs exp.
        fifo = deque()
        markers_done = set()

        def drain(budget_ns):
            while fifo and budget_ns > 0:
                ns, thunk = fifo.popleft()
                thunk()
                budget_ns -= max(ns, 1)

        def drain_until(mid):
            """Pop until the marker `mid` has been consumed.  Guarantees the
            P@V consumers of two-heads-ago P tiles are emitted before their
            ring slots are re-allocated (else the scheduler deadlocks)."""
            if isinstance(mid, int) and mid < 0:
                return
            while mid not in markers_done:
                ns, thunk = fifo.popleft()
                thunk()

        xcs = {}
        qts = {}

        def push_qpass(cn, xcn):
            """Queue the Q projection + rope of chunk cn as PE filler for the
            current chunk's attention phase."""
            qt_n = qpool.tile([128, HPC, CH], BF, tag="qt", name=f"qt{cn}")
            qts[cn] = qt_n
            state = {}

            def mk_q_t(pair, t):
                def f():
                    if t == 0:
                        state[pair] = [
                            ps_acc.tile([128, CH], F32, tag="acc",
                                        name=f"accq{cn}_{pair}_{i}")
                            for i in range(2)]
                    for i in range(2):
                        h = 2 * pair + i
                        nc.tensor.matmul(
                            state[pair][i], wq_sb[:, t, h * DH:(h + 1) * DH],
                            xcn[:, t, :], start=(t == 0), stop=(t == NT - 1),
                            skip_group_check=True)
                return f

            def mk_q_rope(pair):
                def f():
                    for i in range(2):
                        rope(state[pair][i], cn, qt_n[:, 2 * pair + i, :])
                return f

            for pair in range(2):
                for t in range(NT):
                    fifo.append((426, mk_q_t(pair, t)))
                fifo.append((800, mk_q_rope(pair)))
            fifo.append((0, lambda mid=("q", cn): markers_done.add(mid)))

        for c in range(NCHUNK):
            col = c * CH

            # ------------ Q projection ------------
            # Chunk 0: inline single 4-head pass (h2/h3 accumulate in the
            # idle st2 slots) keeping the PE abreast of the initial x/wq DMA
            # stream.  Later chunks were queued as FIFO filler during the
            # previous chunk's attention -- just make sure they are emitted.
            if c == 0:
                xc = xc0
                qt_sb = qpool.tile([128, HPC, CH], BF, tag="qt", name="qt0")
                accs = [ps_acc.tile([128, CH], F32, tag="acc", name=f"accq0_{i}")
                        for i in range(2)]
                accs += [ps_st2.tile([128, CH], F32, tag="st2",
                                     name=f"accq0_{i + 2}") for i in range(2)]
                for t in range(NT):
                    for h in range(HPC):
                        nc.tensor.matmul(
                            accs[h], wq_sb[:, t, h * DH:(h + 1) * DH], xc[:, t, :],
                            start=(t == 0), stop=(t == NT - 1))
                for h in (2, 3, 0, 1):   # free the st2 slots first
                    rope(accs[h], c, qt_sb[:, h, :])
            else:
                xc = xcs[c]
                drain_until(("q", c))
                qt_sb = qts[c]

            # prefetch next chunk's x right away (transfers overlap K/V and
            # attention; the slot's previous readers finished last chunk)
            if c + 1 < NCHUNK:
                xn = xpool.tile([128, NT, CH], BF, tag="xc", name=f"xc{c + 1}")
                xcs[c + 1] = xn
                ncol = (c + 1) * CH
                for i in range(4):
                    nc.sync.dma_start(
                        xn[:, 4 * i:4 * i + 4, :],
                        xT_r[:, 4 * i:4 * i + 4, ncol:ncol + CH])

            # ------------ K,V projections (K first: its rope chain is on
            # the critical path to the diagonal score tiles) ------------
            acc_k = ps_acc.tile([128, CH], F32, tag="acc")
            for t in range(NT):
                nc.tensor.matmul(acc_k, wk_sb[:, t, :], xc[:, t, :],
                                 start=(t == 0), stop=(t == NT - 1))
            rope(acc_k, c, kt_sb[:, col:col + CH])
            acc_v = ps_acc.tile([128, CH], F32, tag="acc")
            for t in range(NT):
                nc.tensor.matmul(acc_v, wv_sb[:, t, :], xc[:, t, :],
                                 start=(t == 0), stop=(t == NT - 1))
            vt_sb = vtp.tile([128, CH], BF, tag="vt")
            nc.scalar.copy(vt_sb, acc_v)
            tpv = ps_st2.tile([128, CH], BF, tag="st2", name=f"tpv{c}")
            for u in range(4):
                nc.tensor.matmul(tpv[:, u * 128:(u + 1) * 128],
                                 vt_sb[:, u * 128:(u + 1) * 128], ident_sb,
                                 is_transpose=True, skip_group_check=True)
            nc.vector.tensor_copy(
                v_sb[:, 4 * c:4 * c + 4, :128],
                tpv.rearrange("p (u n) -> p u n", u=4))

            # queue next chunk's Q projection as attention-phase filler
            if c + 1 < NCHUNK:
                push_qpass(c + 1, xcs[c + 1])

            # ------------ attention ------------
            def emit_scores(h):
                """Score+exp+mask for head h. Returns pslice: j -> (ap, base)
                so the P^T column block for (j, qsub u) is ap[:, base+128u:
                base+128(u+1)]."""
                pslice = {}
                # full key tiles, two per exp instruction
                for j in range(0, 4 * c, 2):
                    st2 = ps_st2.tile([128, 2 * CH], F32, tag="st2",
                                      name=f"st{c}_{h}_{j}")
                    nc.tensor.matmul(st2[:, :CH], kt_sb[:, j * 128:(j + 1) * 128],
                                     qt_sb[:, h, :], start=True, stop=True)
                    nc.tensor.matmul(st2[:, CH:], kt_sb[:, (j + 1) * 128:(j + 2) * 128],
                                     qt_sb[:, h, :], start=True, stop=True)
                    pp = ppool.tile([128, 2 * CH], BF, tag="pp",
                                    name=f"pp{c}_{h}_{j}")
                    nc.scalar.activation(pp, st2, EXP, scale=SCALE)
                    pslice[j] = (pp, 0)
                    pslice[j + 1] = (pp, CH)
                    drain(700)
                # diagonal tiles: t=0,1 packed into [0:896], t=2,3 into
                # [0:384] (suffixes shifted left so the exp input is fully
                # written -- no stale-PSUM regions).
                dj = 4 * c
                st2 = ps_st2.tile([128, 896], F32, tag="st2",
                                  name=f"sd01_{c}_{h}")
                nc.tensor.matmul(st2[:, 0:CH], kt_sb[:, dj * 128:(dj + 1) * 128],
                                 qt_sb[:, h, :], start=True, stop=True)
                nc.tensor.matmul(st2[:, CH:896],
                                 kt_sb[:, (dj + 1) * 128:(dj + 2) * 128],
                                 qt_sb[:, h, 128:], start=True, stop=True)
                pp = ppool.tile([128, 896], BF, tag="pd", name=f"pd01_{c}_{h}",
                                bufs=4)
                nc.scalar.activation(pp, st2, EXP, scale=SCALE)
                nc.vector.tensor_tensor(pp[:, 0:128], pp[:, 0:128], maskd_sb, MULT)
                nc.vector.tensor_tensor(pp[:, CH:CH + 128], pp[:, CH:CH + 128],
                                        maskd_sb, MULT)
                pslice[dj] = (pp, 0)
                pslice[dj + 1] = (pp, CH - 128)   # col = 384 + 128u, u >= 1
                drain(650)
                st2 = ps_st2.tile([128, 384], F32, tag="st2",
                                  name=f"sd23_{c}_{h}")
                nc.tensor.matmul(st2[:, 0:256],
                                 kt_sb[:, (dj + 2) * 128:(dj + 3) * 128],
                                 qt_sb[:, h, 256:], start=True, stop=True)
                nc.tensor.matmul(st2[:, 256:384],
                                 kt_sb[:, (dj + 3) * 128:(dj + 4) * 128],
                                 qt_sb[:, h, 384:], start=True, stop=True)
                pp = ppool.tile([128, 384], BF, tag="pd2", name=f"pd23_{c}_{h}",
                                bufs=4)
                nc.scalar.activation(pp, st2, EXP, scale=SCALE)
                nc.vector.tensor_tensor(pp[:, 0:128], pp[:, 0:128], maskd_sb, MULT)
                nc.vector.tensor_tensor(pp[:, 256:384], pp[:, 256:384],
                                        maskd_sb, MULT)
                pslice[dj + 2] = (pp, -256)       # col = -256 + 128u, u >= 2
                pslice[dj + 3] = (pp, -128)       # col = 256 at u == 3
                drain(450)
                return pslice

            def push_pv(h, pslice, ot_sb):
                """Queue P@V + normalize + transpose for head h."""
                ot0 = ps_ot.tile([128, 2, 132], F32, tag="ot0",
                                 name=f"ot0_{c}_{h}")
                ot1 = ps_ot.tile([128, 2, 132], F32, tag="ot1",
                                 name=f"ot1_{c}_{h}")
                osbs = []

                def mk_mm(u, j, first, last):
                    ot = ot0 if u < 2 else ot1
                    p, base = pslice[j]

                    def f():
                        nc.tensor.matmul(
                            ot[:, u % 2, :129],
                            p[:, base + u * 128:base + (u + 1) * 128],
                            v_sb[:, j, :129],
                            start=first, stop=last, skip_group_check=True)
                    return f

                def mk_div(u):
                    ot = ot0 if u < 2 else ot1

                    def f():
                        rd = rdp.tile([128, 1], F32, tag="rd")
                        nc.vector.reciprocal(rd, ot[:, u % 2, 128:129])
                        osb = osbp.tile([128, 128], BF, tag="osb")
                        nc.vector.tensor_scalar_mul(osb, ot[:, u % 2, :128], rd)
                        osbs.append(osb)
                    return f

                def mk_fin(ot_sb=ot_sb):
                    def f():
                        tp2 = ps_st2.tile([128, CH], BF, tag="st2",
                                          name=f"tp2_{c}_{h}")
                        for u in range(4):
                            nc.tensor.matmul(tp2[:, u * 128:(u + 1) * 128],
                                             osbs[u], ident_sb,
                                             is_transpose=True,
                                             skip_group_check=True)
                        if c == NCHUNK - 1 and h % 2 == 0:
                            nc.scalar.copy(ot_sb[:, h, :], tp2)
                        else:
                            nc.vector.tensor_copy(ot_sb[:, h, :], tp2)
                    return f

                for u in range(4):
                    js = list(range(4 * c + u + 1))
                    for idx, j in enumerate(js):
                        fifo.append(
                            (54, mk_mm(u, j, idx == 0, idx == len(js) - 1)))
                    fifo.append((1, mk_div(u)))
                fifo.append((212, mk_fin()))

            ot_sb = otsb.tile([128, HPC, CH], BF, tag="ot", name=f"ot_sb{c}")
            prev = None
            for h in range(HPC):
                gh = c * HPC + h
                drain_until(gh - 2)
                ps = emit_scores(h)
                if prev is not None:
                    push_pv(h - 1, prev, ot_sb)
                    fifo.append((0, lambda mid=gh - 1: markers_done.add(mid)))
                prev = ps
            push_pv(HPC - 1, prev, ot_sb)
            fifo.append((0, lambda mid=c * HPC + HPC - 1: markers_done.add(mid)))

            # ------------ output projection (queued as filler) ------------
            # One atomic thunk per dt_ tile: the st2 accumulation group must
            # open and close without another st2 alloc slipping in between
            # (the ring has 2 slots; a half-open group would deadlock).
            ystage = ystp.tile([128, NT, CH], BF, tag="yst", name=f"yst{c}")

            def mk_wo(dt_, c=c, ot_sb=ot_sb, ystage=ystage):
                def f():
                    yt_ps = ps_st2.tile([128, CH], F32, tag="st2",
                                        name=f"yt{c}_{dt_}")
                    for h in range(HPC):
                        nc.tensor.matmul(
                            yt_ps, wo_sb[:, h, dt_ * 128:(dt_ + 1) * 128],
                            ot_sb[:, h, :], start=(h == 0), stop=(h == HPC - 1),
                            skip_group_check=True)
                    if c == NCHUNK - 1 and dt_ % 2 == 0:
                        nc.scalar.copy(ystage[:, dt_, :], yt_ps)
                    else:
                        nc.vector.tensor_copy(ystage[:, dt_, :], yt_ps)
                return f

            def mk_store(i, w, c=c, ystage=ystage):
                def f():
                    q = nc.sync if c == NCHUNK - 1 else nc.gpsimd
                    q.dma_start(
                        yT_r[:, w * i:w * (i + 1), c * CH:(c + 1) * CH],
                        ystage[:, w * i:w * (i + 1), :])
                return f

            stw = 1 if c == NCHUNK - 1 else 4
            for dt_ in range(NT):
                fifo.append((852, mk_wo(dt_)))
                if dt_ % stw == stw - 1:
                    fifo.append((1, mk_store(dt_ // stw, stw)))

        drain(1 << 30)

    nc.compile()
    return nc


def _host_prep(x, freqs_cos, freqs_sin, wq, wk, wv, wo):
    """Build the 8 per-core input maps (bf16)."""
    cos_t = np.ascontiguousarray(freqs_cos.T)  # [64, S]
    sin_t = np.ascontiguousarray(freqs_sin.T)
    cs = np.repeat(cos_t, 2, axis=0).astype(np.float32)        # [128, S]
    sn = np.repeat(sin_t, 2, axis=0).astype(np.float32)
    sn[0::2] *= -1.0
    cs = cs.astype(NPBF)
    sn = sn.astype(NPBF)

    pswap = np.zeros((DH, DH), dtype=NPBF)
    idx = np.arange(0, DH, 2)
    pswap[idx, idx + 1] = 1.0
    pswap[idx + 1, idx] = 1.0

    ident = np.eye(128, dtype=NPBF)

    # maskd[jj, z] = 1 where z >= jj: the causal triangle of a diagonal
    # 128x128 block (same for every diagonal tile).
    z = np.arange(128)[None, :]
    jj = np.arange(128)[:, None]
    maskd = (z >= jj).astype(NPBF)

    xTs = [np.ascontiguousarray(x[b].T).astype(NPBF) for b in range(2)]
    wq16 = wq.astype(NPBF)
    wk16 = wk.astype(NPBF)
    wv16 = wv.astype(NPBF)
    wo16 = wo.astype(NPBF)

    in_maps = []
    for core in range(N_CORES):
        b, g = divmod(core, HPC)
        in_maps.append({
            "xT": xTs[b],
            "wq": np.ascontiguousarray(wq16[:, g * HPC * DH:(g + 1) * HPC * DH]),
            "wk": np.ascontiguousarray(wk16[:, g * DH:(g + 1) * DH]),
            "wv": np.ascontiguousarray(wv16[:, g * DH:(g + 1) * DH]),
            "wo": np.ascontiguousarray(wo16[g * HPC * DH:(g + 1) * HPC * DH, :]),
            "cs": cs, "sn": sn, "pswap": pswap, "ident": ident, "maskd": maskd,
        })
    return in_maps


def kernel(x, freqs_cos, freqs_sin, mask, wq, wk, wv, wo):
    x = np.asarray(x, dtype=np.float32)
    freqs_cos = np.asarray(freqs_cos, dtype=np.float32)
    freqs_sin = np.asarray(freqs_sin, dtype=np.float32)
    wq = np.asarray(wq, dtype=np.float32)
    wk = np.asarray(wk, dtype=np.float32)
    wv = np.asarray(wv, dtype=np.float32)
    wo = np.asarray(wo, dtype=np.float32)

    if "nc" not in _CACHE:
        _CACHE["nc"] = _build()
    nc = _CACHE["nc"]

    in_maps = _host_prep(x, freqs_cos, freqs_sin, wq, wk, wv, wo)
    res = bass_utils.run_bass_kernel_spmd(nc, in_maps, core_ids=list(range(N_CORES)))

    out = np.empty((2, S, DM), dtype=np.float32)
    for b in range(2):
        acc = res.results[b * HPC]["yT"].astype(np.float32)
        for g in range(1, HPC):
            acc = acc + res.results[b * HPC + g]["yT"].astype(np.float32)
        out[b] = acc.T
    return out
